# revision 5
# baseline (speedup 1.0000x reference)
"""Trainium2 Bass kernel: 8-layer ternary (BitNet-1.58) dense transformer.

Model (per reference):
    h = embed[input_ids]                                  # (B=2, S=1024, H=2048)
    8x: y = h @ ternary(W_l)^T + b_l ; h = LN(y + h)*g+b  # H=2048
    h = LN(h)*final_g + final_b
    logits = h @ ternary(head_W)^T                        # (B, S, V=32000)

Sharding over 8 NeuronCores:
  - Layers: data-parallel over the 2048 tokens (256 tokens/core). Each core
    streams the full ternary layer weights (fp8, exact); no collectives.
  - Head: 8-way tensor-parallel over vocab (4000 vocab rows/core). Final
    hidden states are exchanged with two AllGathers (one per 128-token tile,
    so the first overlaps the last tile's LN tail); each core then computes
    all 2048 tokens x its vocab shard and the host concatenates.

Engine assignment per layer step (128-token tile):
  TensorE: 16 bf16 transposes (bf16 identity -> 1 cyc/row) + 64 matmuls
           (bf16 stationary hT x fp8 ternary moving weights).
  VectorE: fused residual-add+row-sum (tensor_tensor_reduce) + LN stats
           + the (z-mean)*rstd normalize (writes bf16 state).
  ScalarE: Square with accum (sum-of-squares) + Sqrt.
  PoolE:   PSUM->SBUF transpose-cast with the ternary scale folded in.
"""

import os
import sys

import numpy as np

try:
    import concourse.bass as bass
except ImportError:  # grading container should have it on sys.path already
    sys.path.insert(0, "/opt/trn_rl_repo")
    import concourse.bass as bass

import ml_dtypes
import concourse.mybir as mybir
import concourse.tile as tile
from concourse import bacc
from concourse.bass_utils import run_bass_kernel_spmd
from contextlib import ExitStack

F32 = mybir.dt.float32
BF16 = mybir.dt.bfloat16
FP8 = mybir.dt.float8e4
AX = mybir.AxisListType
OP = mybir.AluOpType
AF = mybir.ActivationFunctionType
EPS = 1e-5

# Feature flags (validated by microbenches; flip off if hardware disagrees)
W8 = True         # ternary weights shipped/loaded as fp8 (exact for {-1,0,1})
POOL_CAST = False  # Pool engine cannot read PSUM (compile fails)
USE_STT = True    # fused residual-add + row-sum via scalar_tensor_tensor

# Full-size problem config (B=2, S=1024 -> 2048 tokens).
CFG_FULL = dict(L=8, H=2048, NTOK=2048, NC=8, TT=2, VS=4000, CH=512, HCH=500)

WDT = FP8 if W8 else BF16


def build_nc(cfg, scales, head_scale, trivial):
    L, H, NTOK, NC, TT = cfg["L"], cfg["H"], cfg["NTOK"], cfg["NC"], cfg["TT"]
    VS, CH, HCH = cfg["VS"], cfg["CH"], cfg["HCH"]
    KT = H // 128
    NCH = H // CH
    NHV = VS // HCH
    assert NTOK == NC * TT * 128

    nc = bacc.Bacc("TRN2", target_bir_lowering=False, debug=False, num_devices=NC)
    h0 = nc.declare_dram_parameter("h0", [TT, 128, H], BF16, isOutput=False)
    w_ = nc.declare_dram_parameter("w", [L, KT, 128, H], WDT, isOutput=False)
    hw_ = nc.declare_dram_parameter("hw", [KT, 128, VS], WDT, isOutput=False)
    ident_d = nc.declare_dram_parameter("ident", [128, 128], BF16, isOutput=False)
    eps_d = nc.declare_dram_parameter("eps", [128, 1], F32, isOutput=False)
    if not trivial:
        lng = nc.declare_dram_parameter("lng", [L, H], BF16, isOutput=False)
        lnb = nc.declare_dram_parameter("lnb", [L, H], BF16, isOutput=False)
        lbias = nc.declare_dram_parameter("lbias", [L, H], BF16, isOutput=False)
        fing = nc.declare_dram_parameter("fing", [H], BF16, isOutput=False)
        finb = nc.declare_dram_parameter("finb", [H], BF16, isOutput=False)
    out = nc.declare_dram_parameter("out", [NTOK, VS], F32, isOutput=True)
    hT_loc = [nc.dram_tensor(f"hT_loc{t}", [128, KT, 128], BF16) for t in range(TT)]
    hT_all = [
        nc.dram_tensor(
            f"hT_all{t}",
            [NC, 128, KT, 128],
            BF16,
            addr_space="Shared" if NC > 4 else "Local",
        )
        for t in range(TT)
    ]

    with tile.TileContext(nc) as tc:
        with ExitStack() as ctx0:
            consts = ctx0.enter_context(tc.tile_pool(name="consts", bufs=1))
            hTfinp = ctx0.enter_context(tc.tile_pool(name="hTfin", bufs=TT))
            headwp = ctx0.enter_context(tc.tile_pool(name="headw", bufs=1))

            ident = consts.tile([128, 128], BF16)
            nc.sync.dma_start(ident[:], ident_d[:])
            eps_t = consts.tile([128, 1], F32)
            nc.sync.dma_start(eps_t[:], eps_d[:])

            hw_t = headwp.tile([128, KT, VS], WDT)

            hT_store = []  # per-t store instruction for collective deps
            hT_fin = [None] * TT

            with ExitStack() as ctxA:
                state = ctxA.enter_context(tc.tile_pool(name="state", bufs=4))
                zp = ctxA.enter_context(tc.tile_pool(name="z", bufs=2))
                sqp = ctxA.enter_context(tc.tile_pool(name="sq", bufs=2))
                hTp = ctxA.enter_context(tc.tile_pool(name="hT", bufs=2))
                wp = ctxA.enter_context(tc.tile_pool(name="w", bufs=2))
                smp = ctxA.enter_context(tc.tile_pool(name="small", bufs=16))
                psT = ctxA.enter_context(
                    tc.tile_pool(name="psT", bufs=1, space="PSUM")
                )
                psY = ctxA.enter_context(
                    tc.tile_pool(name="psY", bufs=6, space="PSUM")
                )
                if not trivial:
                    gbp = ctxA.enter_context(tc.tile_pool(name="gb", bufs=2))

                h_cur = []
                for t in range(TT):
                    st = state.tile([128, H], BF16, name=f"hinit{t}", tag="state")
                    nc.sync.dma_start(st[:], h0[t])
                    h_cur.append(st)

                def transpose_cast(src_bf16, scale_imm, name):
                    """h [128tok, H] bf16 -> hT [128feat, (kt,128tok)] bf16 * s."""
                    pT = psT.tile([128, H], BF16, tag="psT", name=f"psT{name}")
                    for kt in range(KT):
                        nc.tensor.transpose(
                            pT[:, kt * 128 : (kt + 1) * 128],
                            src_bf16[:, kt * 128 : (kt + 1) * 128],
                            ident[:],
                        )
                    dst = hTp.tile([128, H], BF16, tag="hT", name=f"hT{name}")
                    if POOL_CAST:
                        nc.gpsimd.tensor_scalar_mul(dst[:], pT[:], float(scale_imm))
                    else:
                        nc.scalar.activation(
                            dst[:], pT[:], AF.Copy, scale=float(scale_imm)
                        )
                    return dst

                def ln_stats_finish(z_src, S4, SS4, nch, name, g_t=None, b_t=None):
                    """Given per-chunk sums S4 [128,nch] / sq-sums SS4, produce
                    normalized bf16 state tile."""
                    S = smp.tile([128, 1], F32, tag="s0", name=f"S{name}")
                    nc.vector.tensor_reduce(S[:], S4[:], axis=AX.X, op=OP.add)
                    SS = smp.tile([128, 1], F32, tag="s1", name=f"SS{name}")
                    nc.vector.tensor_reduce(SS[:], SS4[:], axis=AX.X, op=OP.add)
                    negmean = smp.tile([128, 1], F32, tag="s2", name=f"nm{name}")
                    nc.vector.tensor_scalar_mul(negmean[:], S[:], -1.0 / H)
                    msq = smp.tile([128, 1], F32, tag="s3", name=f"msq{name}")
                    nc.vector.tensor_scalar_mul(msq[:], SS[:], 1.0 / H)
                    var = smp.tile([128, 1], F32, tag="s4", name=f"var{name}")
                    nc.vector.tensor_tensor(var[:], negmean[:], negmean[:], OP.mult)
                    nc.vector.tensor_tensor(var[:], msq[:], var[:], OP.subtract)
                    std = smp.tile([128, 1], F32, tag="s5", name=f"std{name}")
                    nc.scalar.activation(std[:], var[:], AF.Sqrt, bias=eps_t[:])
                    rstd = smp.tile([128, 1], F32, tag="s6", name=f"rstd{name}")
                    nc.vector.reciprocal(rstd[:], std[:])
                    hn = state.tile([128, H], BF16, tag="state", name=f"h{name}")
                    nc.vector.tensor_scalar(
                        hn[:], z_src[:], negmean[:], rstd[:], OP.add, OP.mult
                    )
                    if g_t is not None:
                        nc.gpsimd.tensor_tensor(hn[:], hn[:], g_t[:], OP.mult)
                        nc.gpsimd.tensor_tensor(hn[:], hn[:], b_t[:], OP.add)
                    return hn

                for l in range(L):
                    wt = wp.tile([128, KT, H], WDT, tag="w", name=f"w{l}")
                    nc.sync.dma_start(
                        wt[:], w_[l].rearrange("k p o -> p k o")
                    )
                    if l == 0:
                        # head weights prefetch: queued after layer-0 weights
                        nc.sync.dma_start(
                            hw_t[:], hw_.rearrange("k p v -> p k v")
                        )
                    g_t = b_t = bias_t = None
                    if not trivial:
                        g_t = gbp.tile([128, H], BF16, tag="g", name=f"g{l}")
                        nc.sync.dma_start(
                            g_t[:], lng[l][None, :].to_broadcast((128, H))
                        )
                        b_t = gbp.tile([128, H], BF16, tag="b", name=f"b{l}")
                        nc.sync.dma_start(
                            b_t[:], lnb[l][None, :].to_broadcast((128, H))
                        )
                        bias_t = gbp.tile([128, H], BF16, tag="bias", name=f"bias{l}")
                        nc.sync.dma_start(
                            bias_t[:], lbias[l][None, :].to_broadcast((128, H))
                        )

                    for t in range(TT):
                        hTt = transpose_cast(h_cur[t], scales[l], f"{l}_{t}")
                        resid = h_cur[t]
                        if not trivial:
                            hb = zp.tile([128, H], BF16, tag="hb", name=f"hb{l}_{t}")
                            nc.gpsimd.tensor_tensor(
                                hb[:], h_cur[t][:], bias_t[:], OP.add
                            )
                            resid = hb
                        z = zp.tile([128, H], F32, tag="z", name=f"z{l}_{t}")
                        S4 = smp.tile([128, NCH], F32, tag="s7", name=f"S4_{l}_{t}")
                        SS4 = smp.tile([128, NCH], F32, tag="s8", name=f"SS4_{l}_{t}")
                        for i in range(NCH):
                            sl = slice(i * CH, (i + 1) * CH)
                            p = psY.tile([128, CH], F32, tag="psY", name=f"ps{l}_{t}_{i}")
                            for kt in range(KT):
                                nc.tensor.matmul(
                                    p[:],
                                    lhsT=hTt[:, kt * 128 : (kt + 1) * 128],
                                    rhs=wt[:, kt, sl],
                                    start=(kt == 0),
                                    stop=(kt == KT - 1),
                                )
                            if USE_STT:
                                nc.vector.scalar_tensor_tensor(
                                    z[:, sl], p[:], 0.0, resid[:, sl],
                                    OP.add, OP.add, accum_out=S4[:, i : i + 1],
                                )
                            else:
                                nc.vector.tensor_add(z[:, sl], p[:], resid[:, sl])
                                nc.vector.tensor_reduce(
                                    S4[:, i : i + 1], z[:, sl], axis=AX.X, op=OP.add
                                )
                            sq = sqp.tile([128, CH], F32, tag="sq", name=f"sq{l}_{t}_{i}")
                            nc.scalar.activation(
                                sq[:], z[:, sl], AF.Square,
                                accum_out=SS4[:, i : i + 1],
                            )
                        h_cur[t] = ln_stats_finish(
                            z, S4, SS4, NCH, f"{l}_{t}", g_t, b_t
                        )

                # final LN + head-input transposes; one collective per t so the
                # first gather overlaps the second tile's tail.
                fg = fb = None
                if not trivial:
                    fg = gbp.tile([128, H], BF16, tag="g", name="gfin")
                    nc.sync.dma_start(fg[:], fing[None, :].to_broadcast((128, H)))
                    fb = gbp.tile([128, H], BF16, tag="b", name="bfin")
                    nc.sync.dma_start(fb[:], finb[None, :].to_broadcast((128, H)))
                for t in range(TT):
                    h8 = h_cur[t]
                    S4 = smp.tile([128, NCH], F32, tag="s7", name=f"S4f{t}")
                    SS4 = smp.tile([128, NCH], F32, tag="s8", name=f"SS4f{t}")
                    for i in range(NCH):
                        sl = slice(i * CH, (i + 1) * CH)
                        nc.vector.tensor_reduce(
                            S4[:, i : i + 1], h8[:, sl], axis=AX.X, op=OP.add
                        )
                        sq = sqp.tile([128, CH], F32, tag="sq", name=f"sqf{t}_{i}")
                        nc.scalar.activation(
                            sq[:], h8[:, sl], AF.Square, accum_out=SS4[:, i : i + 1]
                        )
                    hfin = ln_stats_finish(h8, S4, SS4, NCH, f"fin{t}", fg, fb)
                    # transpose+scale into a tile that survives into the head
                    pT = psT.tile([128, H], BF16, tag="psT", name=f"psTfin{t}")
                    for kt in range(KT):
                        nc.tensor.transpose(
                            pT[:, kt * 128 : (kt + 1) * 128],
                            hfin[:, kt * 128 : (kt + 1) * 128],
                            ident[:],
                        )
                    hTf = hTfinp.tile([128, H], BF16, name=f"hTfin{t}")
                    if POOL_CAST:
                        nc.gpsimd.tensor_scalar_mul(hTf[:], pT[:], float(head_scale))
                    else:
                        nc.scalar.activation(
                            hTf[:], pT[:], AF.Copy, scale=float(head_scale)
                        )
                    hT_fin[t] = hTf
                    st_i = nc.sync.dma_start(
                        hT_loc[t][:], hTf[:].rearrange("p (k u) -> p k u", k=KT)
                    )
                    hT_store.append(st_i)

            # collectives (one per token tile)
            ccs = []
            for t in range(TT):
                cc = nc.gpsimd.collective_compute(
                    "AllGather",
                    OP.bypass,
                    replica_groups=[list(range(NC))],
                    ins=[hT_loc[t][:]],
                    outs=[hT_all[t][:]],
                )
                tile.add_dep_helper(
                    cc.ins, hT_store[t].ins, sync=True,
                    reason=f"gather{t} waits on hT store{t}",
                )
                ccs.append(cc)

            # head phase: all 2048 tokens x this core's 4000-vocab shard
            with ExitStack() as ctxB:
                hTsp = ctxB.enter_context(tc.tile_pool(name="hTs", bufs=4))
                outp = ctxB.enter_context(tc.tile_pool(name="outstg", bufs=6))
                psH = ctxB.enter_context(
                    tc.tile_pool(name="psH", bufs=4, space="PSUM")
                )

                for t in range(TT):
                    for c in range(NC):
                        hTs = hTsp.tile(
                            [128, KT, 128], BF16, tag="hTs", name=f"hTs{t}_{c}"
                        )
                        ld = nc.sync.dma_start(hTs[:], hT_all[t][c])
                        tile.add_dep_helper(
                            ld.ins, ccs[t].ins, sync=True,
                            reason="hT load waits on gather",
                        )
                        row0 = (c * TT + t) * 128
                        for v in range(NHV):
                            vsl = slice(v * HCH, (v + 1) * HCH)
                            p = psH.tile(
                                [128, HCH], F32, tag="psH", name=f"ph{t}_{c}_{v}"
                            )
                            for kt in range(KT):
                                nc.tensor.matmul(
                                    p[:],
                                    lhsT=hTs[:, kt, :],
                                    rhs=hw_t[:, kt, vsl],
                                    start=(kt == 0),
                                    stop=(kt == KT - 1),
                                )
                            o_t = outp.tile(
                                [128, HCH], F32, tag="ostg", name=f"o{t}_{c}_{v}"
                            )
                            nc.scalar.copy(o_t[:], p[:])
                            nc.sync.dma_start(
                                out[row0 : row0 + 128, vsl], o_t[:]
                            )

    return nc


def _ternary(wmat):
    """Exact {-1,0,1} ternary tensor + fp32 scale, matching the reference."""
    w = np.asarray(wmat, dtype=np.float32)
    s = np.mean(np.abs(w), dtype=np.float32)
    t = np.clip(np.rint(w / (s + np.float32(1e-8))), -1.0, 1.0).astype(np.float32)
    return t, float(s)


_NC_CACHE = {}
_LAST_RESULTS = None


def kernel(**inputs):
    global _LAST_RESULTS
    cfg = CFG_FULL
    L, H, NTOK, NC, TT, VS = (
        cfg["L"], cfg["H"], cfg["NTOK"], cfg["NC"], cfg["TT"], cfg["VS"],
    )
    KT = H // 128
    TPC = TT * 128  # tokens per core
    BF = ml_dtypes.bfloat16
    W8NP = ml_dtypes.float8_e4m3 if W8 else BF

    ids = np.asarray(inputs["input_ids"]).astype(np.int64).reshape(-1)
    embed = np.asarray(inputs["embed"], dtype=np.float32)
    layer_w = np.asarray(inputs["layer_w"], dtype=np.float32)
    layer_b = np.asarray(inputs["layer_b"], dtype=np.float32)
    ln_g = np.asarray(inputs["ln_g"], dtype=np.float32)
    ln_b = np.asarray(inputs["ln_b"], dtype=np.float32)
    final_g = np.asarray(inputs["final_g"], dtype=np.float32)
    final_b = np.asarray(inputs["final_b"], dtype=np.float32)
    head_w = np.asarray(inputs["head_w"], dtype=np.float32)

    trivial = bool(
        np.all(ln_g == 1.0) and np.all(ln_b == 0.0) and np.all(layer_b == 0.0)
        and np.all(final_g == 1.0) and np.all(final_b == 0.0)
    )

    h0_full = embed[ids]  # [NTOK, H] fp32

    scales = []
    wT = np.empty([L, KT, 128, H], dtype=W8NP)
    for l in range(L):
        t, s = _ternary(layer_w[l])
        scales.append(s)
        wT[l] = np.ascontiguousarray(t.T).reshape(KT, 128, H).astype(W8NP)
    th, head_scale = _ternary(head_w)
    headT = np.ascontiguousarray(th.T).astype(W8NP)  # [H, V]

    key = (id(cfg), tuple(scales), head_scale, trivial)
    if key not in _NC_CACHE:
        _NC_CACHE.clear()
        nc = build_nc(cfg, scales, head_scale, trivial)
        nc.finalize()
        _NC_CACHE[key] = nc
    nc = _NC_CACHE[key]

    common = {
        "w": wT,
        "ident": np.eye(128, dtype=BF),
        "eps": np.full((128, 1), EPS, np.float32),
    }
    if not trivial:
        common.update(
            lng=ln_g.astype(BF),
            lnb=ln_b.astype(BF),
            lbias=layer_b.astype(BF),
            fing=final_g.astype(BF),
            finb=final_b.astype(BF),
        )
    in_maps = []
    for c in range(NC):
        in_maps.append(
            dict(
                common,
                h0=np.ascontiguousarray(
                    h0_full[c * TPC : (c + 1) * TPC].reshape(TT, 128, H)
                ).astype(BF),
                hw=np.ascontiguousarray(
                    headT[:, c * VS : (c + 1) * VS].reshape(KT, 128, VS)
                ),
            )
        )

    trace = bool(int(os.environ.get("TRIKERNEL_TRACE", "0")))
    res = run_bass_kernel_spmd(nc, in_maps, core_ids=list(range(NC)), trace=trace)
    _LAST_RESULTS = res

    full = np.concatenate(
        [np.asarray(res.results[c]["out"]) for c in range(NC)], axis=1
    )  # [NTOK, V]
    return full.reshape(2, 1024, 32000).astype(np.float32)
